# revision 8
# baseline (speedup 1.0000x reference)
"""Trainium2 Bass kernel for CRF log-likelihood (B=128, S=512, U=1024, T=48).

Strategy (data-parallel, 16 batch rows per core, no collectives):
  - Emissions scores = H @ W computed on PE (K=1024 in 8 chunks of 128),
    H streamed from HBM with U on partitions (fully contiguous reads).
  - Forward algorithm in exp space: one (49x49)@(49x16) PE matmul + one
    DVE multiply per time step.  A 49th "done" state absorbs finished rows
    (transition column = exp(end_transitions)), driven purely by per-core
    data masks, so all cores run the identical SPMD program.
  - A constant per-step normalizer exp(-C0) keeps fp32 in range; corrected
    on the host via + C0*(s_len-1).
  - The chain is split into a forward scan (steps 1..255) and an
    independent backward scan (steps 511..256) that run concurrently,
    halving the sequential latency.  Z = sum_j alpha_cut[j]*beta_cut[j].
  - Gold-path emission sum (numerator) on device via a host-built
    onehot*mask multiply + reduce against the same score tiles.
  - Tiny O(B*S) gathers of the small parameter tensors (transition/start/
    end terms of the numerator, final log/assembly) happen on the host.
"""

import os

import numpy as np

import concourse.bass as bass
import concourse.tile as tile
from concourse import bacc, mybir
from concourse.bass_utils import run_bass_kernel_spmd

B, S, U, T = 128, 512, 1024, 48
NCORES = 8
NB = B // NCORES          # 16 rows per core
NPOS = NB * S             # 8192 positions per core, pos = s*NB + b
TA = T + 1                # 49 states (48 tags + "done")
CUT = 255                 # fwd computes alpha_1..alpha_CUT, bwd beta_510..beta_CUT
C0 = 4.8                  # per-step log-space normalizer
SCHUNK = 64               # time steps per emission chunk
NCHUNK = S // SCHUNK      # 8
CPOS = SCHUNK * NB        # 1024 positions per chunk -> 2 PSUM halves of 512
NEG = -1.0e9              # pad logit; exp(NEG) == 0 in fp32
F32 = mybir.dt.float32

_PROGRAM = None  # compiled program cache
LAST_EXEC_NS = None
LAST_RESULT = None


def _build_program():
    nc = bacc.Bacc("TRN2", target_bir_lowering=False, debug=False,
                   enable_asserts=False)

    def din(name, shape):
        return nc.dram_tensor(name, list(shape), F32, kind="ExternalInput").ap()

    def dout(name, shape):
        return nc.dram_tensor(name, list(shape), F32, kind="ExternalOutput").ap()

    h = din("h", (U, S, NB))  # host-pretransposed
    w = din("w", (U, TA))  # 49th col zero
    lhs_fwd = din("lhs_fwd", (TA, TA))      # Ahat: [i, j]
    lhs_bwd = din("lhs_bwd", (TA, TA))      # Ahat^T
    ones_k1 = din("ones_k1", (1, TA))  # [1]*48 + [-1]
    ones49 = din("ones49", (TA, 1))
    padflag = din("padflag", (1, NPOS))     # {0, NEG}
    msel = din("msel", (TA, NPOS))          # onehot(tag)*wmask, row48=0
    bias_e = din("bias_e", (TA, 1))         # [b - C0; NEG]
    bias_a0 = din("bias_a0", (TA, 1))       # [b + start; NEG]
    beta_init = din("beta_init", (TA, NB))  # [exp(end); 1]

    z_out = dout("z_out", (1, NB))
    numstage_out = dout("numstage", (TA, 2 * NCHUNK * NB))

    with tile.TileContext(nc) as tc:
        with (
            tc.tile_pool(name="consts", bufs=1) as consts,
            tc.tile_pool(name="hpool", bufs=2) as hpool,
            tc.tile_pool(name="tmp", bufs=2) as tmpp,
            tc.tile_pool(name="epsum", bufs=2, space="PSUM") as epsum,
            tc.tile_pool(name="psA", bufs=2, space="PSUM") as psA,
            tc.tile_pool(name="psB", bufs=2, space="PSUM") as psB,
            tc.tile_pool(name="psZ", bufs=1, space="PSUM") as psZ,
            tc.tile_pool(name="sA", bufs=2) as sAp,
            tc.tile_pool(name="sB", bufs=2) as sBp,
        ):
            # ---- constants into SBUF ----
            w_sb = consts.tile([128, 8 * TA], F32, tag="w_sb")
            nc.sync.dma_start(w_sb[:].rearrange("p (c t) -> p c t", c=8),
                              w.rearrange("(c p) t -> p c t", p=128))
            lhsf_sb = consts.tile([TA, TA], F32, tag="lhsf")
            nc.sync.dma_start(lhsf_sb[:], lhs_fwd)
            lhsb_sb = consts.tile([TA, TA], F32, tag="lhsb")
            nc.sync.dma_start(lhsb_sb[:], lhs_bwd)
            ones1_sb = consts.tile([1, TA], F32, tag="ones1")
            nc.sync.dma_start(ones1_sb[:], ones_k1)
            ones49_sb = consts.tile([TA, 1], F32, tag="ones49v")
            nc.sync.dma_start(ones49_sb[:], ones49)
            pad_sb = consts.tile([1, NPOS], F32, tag="pad")
            nc.sync.dma_start(pad_sb[:], padflag)
            msel_sb = consts.tile([TA, NPOS], F32, tag="msel")
            nc.sync.dma_start(msel_sb[:], msel)
            bias_e_sb = consts.tile([TA, 1], F32, tag="bias_e")
            nc.sync.dma_start(bias_e_sb[:], bias_e)
            bias_a0_sb = consts.tile([TA, 1], F32, tag="bias_a0")
            nc.sync.dma_start(bias_a0_sb[:], bias_a0)
            beta0_sb = consts.tile([TA, NB], F32, tag="beta0")
            nc.sync.dma_start(beta0_sb[:], beta_init)

            escan = consts.tile([TA, NPOS], F32, tag="escan")
            numstage_sb = consts.tile([TA, 2 * NCHUNK * NB], F32, tag="numstage")
            alpha0_sb = consts.tile([TA, NB], F32, tag="alpha0")

            def produce_chunk(c):
                hs = hpool.tile([128, CPOS * 8], F32, tag="hs")
                for hh in range(8):
                    src = h[hh * 128:(hh + 1) * 128,
                            c * SCHUNK:(c + 1) * SCHUNK, :].rearrange(
                        "p s b -> p (s b)")
                    nc.sync.dma_start(hs[:, hh * CPOS:(hh + 1) * CPOS], src)
                for half in range(2):
                    pos0 = c * CPOS + half * 512
                    ps = epsum.tile([TA, 512], F32, tag="eps")
                    for hh in range(8):
                        nc.tensor.matmul(
                            ps[:],
                            w_sb[:, hh * TA:(hh + 1) * TA],
                            hs[:, hh * CPOS + half * 512: hh * CPOS + half * 512 + 512],
                            start=(hh == 0), stop=False)
                    nc.tensor.matmul(ps[:], ones1_sb[:],
                                     pad_sb[:, pos0:pos0 + 512],
                                     start=False, stop=True)
                    nc.scalar.activation(escan[:, pos0:pos0 + 512], ps[:],
                                         mybir.ActivationFunctionType.Exp,
                                         bias=bias_e_sb[:])
                    if c == 0 and half == 0:
                        nc.scalar.activation(alpha0_sb[:], ps[:, 0:NB],
                                             mybir.ActivationFunctionType.Exp,
                                             bias=bias_a0_sb[:])
                    t_ = tmpp.tile([TA, 512], F32, tag="nmul")
                    nc.vector.tensor_tensor(t_[:], ps[:],
                                            msel_sb[:, pos0:pos0 + 512],
                                            mybir.AluOpType.mult)
                    nc.vector.tensor_reduce(
                        numstage_sb[:, (2 * c + half) * NB:(2 * c + half + 1) * NB],
                        t_[:].rearrange("t (s b) -> t b s", b=NB),
                        mybir.AxisListType.X, mybir.AluOpType.add)

            produce_chunk(0)
            produce_chunk(NCHUNK - 1)

            # ---- the two scan chains, interleaved ----
            alpha = alpha0_sb
            beta = beta0_sb
            prefetch = {}
            for c in range(1, 4):
                lead = c * SCHUNK - 40
                prefetch.setdefault(lead, []).extend([c, NCHUNK - 1 - c])

            for i in range(CUT):
                for c in prefetch.get(i, ()):
                    produce_chunk(c)
                s_f = 1 + i
                pa = psA.tile([TA, NB], F32, tag="pa")
                nc.tensor.matmul(pa[:], lhsf_sb[:], alpha[:], start=True, stop=True)
                na = sAp.tile([TA, NB], F32, tag="na")
                nc.vector.tensor_tensor(na[:], pa[:],
                                        escan[:, s_f * NB:(s_f + 1) * NB],
                                        mybir.AluOpType.mult)
                alpha = na

                s_b = S - 1 - i
                rb = sBp.tile([TA, NB], F32, tag="rb")
                nc.vector.tensor_tensor(rb[:], beta[:],
                                        escan[:, s_b * NB:(s_b + 1) * NB],
                                        mybir.AluOpType.mult)
                pb = psB.tile([TA, NB], F32, tag="pb")
                nc.tensor.matmul(pb[:], lhsb_sb[:], rb[:], start=True, stop=True)
                beta = pb

            # final bwd step: s_b = CUT+1 = 256 -> beta_255
            rb = sBp.tile([TA, NB], F32, tag="rb")
            nc.vector.tensor_tensor(rb[:], beta[:],
                                    escan[:, (CUT + 1) * NB:(CUT + 2) * NB],
                                    mybir.AluOpType.mult)
            pb = psB.tile([TA, NB], F32, tag="pb")
            nc.tensor.matmul(pb[:], lhsb_sb[:], rb[:], start=True, stop=True)

            # ---- readout: z = sum_j alpha_cut[j] * beta_cut[j] ----
            g = sAp.tile([TA, NB], F32, tag="gamma")
            nc.vector.tensor_tensor(g[:], pb[:], alpha[:], mybir.AluOpType.mult)
            zp = psZ.tile([1, NB], F32, tag="zp")
            nc.tensor.matmul(zp[:], ones49_sb[:], g[:], start=True, stop=True)
            zsb = consts.tile([1, NB], F32, tag="zsb")
            nc.vector.tensor_copy(zsb[:], zp[:])
            nc.sync.dma_start(z_out, zsb[:])
            nc.sync.dma_start(numstage_out, numstage_sb[:])

    nc.compile()
    return nc


def _host_inputs(H, W, bb, st, en, tr, tag, s_len, w_mask):
    """Build the per-core input maps (all f32)."""
    A = np.exp(tr.astype(np.float64)).astype(np.float32)
    Ahat = np.zeros((TA, TA), np.float32)
    Ahat[:T, :T] = A
    Ahat[:T, T] = np.exp(en).astype(np.float32)
    Ahat[T, T] = 1.0

    beta_init = np.zeros((TA, NB), np.float32)
    beta_init[:T, :] = np.exp(en).astype(np.float32)[:, None]
    beta_init[T, :] = 1.0

    Wp = np.zeros((U, TA), np.float32)
    Wp[:, :T] = W
    ones_k1 = np.ones((1, TA), np.float32)
    ones_k1[0, T] = -1.0
    shared = {
        "w": Wp,
        "lhs_fwd": Ahat,
        "lhs_bwd": np.ascontiguousarray(Ahat.T),
        "ones_k1": ones_k1,
        "ones49": np.ones((TA, 1), np.float32),
        "bias_e": np.concatenate([(bb - C0).astype(np.float32),
                                  [np.float32(NEG)]]).reshape(TA, 1),
        "bias_a0": np.concatenate([(bb + st).astype(np.float32),
                                   [np.float32(NEG)]]).reshape(TA, 1),
        "beta_init": beta_init,
    }

    s_idx = np.arange(S)
    in_maps = []
    for k in range(NCORES):
        rows = slice(k * NB, (k + 1) * NB)
        tag_l = tag[rows]            # (NB, S)
        len_l = s_len[rows]          # (NB,)
        wm_l = w_mask[rows]          # (NB, S)
        pad = (s_idx[None, :] >= len_l[:, None])          # (NB, S)
        padflag = np.where(pad, np.float32(NEG), np.float32(0.0)).T.reshape(1, NPOS)
        msel3 = np.zeros((TA, S, NB), np.float32)
        msel3[tag_l.T, s_idx[:, None], np.arange(NB)[None, :]] = wm_l.T
        im = dict(shared)
        im["h"] = np.ascontiguousarray(H[rows].transpose(2, 1, 0))
        im["padflag"] = np.ascontiguousarray(padflag)
        im["msel"] = np.ascontiguousarray(msel3.reshape(TA, NPOS))
        in_maps.append(im)
    return in_maps


def kernel(H, W, b, start_transitions, end_transitions, transitions,
           tag, s_len, w_mask):
    global _PROGRAM
    H = np.asarray(H, np.float32)
    W = np.asarray(W, np.float32)
    bb = np.asarray(b, np.float32)
    st = np.asarray(start_transitions, np.float32)
    en = np.asarray(end_transitions, np.float32)
    tr = np.asarray(transitions, np.float32)
    tag = np.asarray(tag)
    s_len = np.asarray(s_len)
    w_mask = np.asarray(w_mask, np.float32)

    if _PROGRAM is None:
        _PROGRAM = _build_program()
    nc = _PROGRAM

    in_maps = _host_inputs(H, W, bb, st, en, tr, tag, s_len, w_mask)
    trace = bool(int(os.environ.get("KERNEL_TRACE", "0")))
    r = run_bass_kernel_spmd(nc, in_maps, list(range(NCORES)), trace=trace,
                             tmpdir=os.environ.get("KERNEL_TRACE_DIR") or None)
    global LAST_EXEC_NS, LAST_RESULT
    LAST_RESULT = r
    LAST_EXEC_NS = r.exec_time_ns
    res = r.results

    z = np.concatenate([np.asarray(r["z_out"]).reshape(NB) for r in res])
    numstage = np.stack([np.asarray(r["numstage"]) for r in res])  # (NC, TA, 32*NB)

    # ---- host assembly ----
    logZ = np.log(z.astype(np.float64)) + C0 * (s_len.astype(np.float64) - 1)
    num_emit = (numstage.reshape(NCORES, TA, 2 * NCHUNK, NB).sum(axis=(1, 2))
                .reshape(B).astype(np.float64))
    bidx = np.arange(B)
    num = (st[tag[:, 0]].astype(np.float64)
           + num_emit
           + (bb[tag].astype(np.float64) * w_mask).sum(axis=1)
           + (tr[tag[:, :-1], tag[:, 1:]].astype(np.float64) * w_mask[:, 1:]).sum(axis=1)
           + en[tag[bidx, s_len - 1]].astype(np.float64))
    return (num - logZ).astype(np.float32)


# revision 12
# speedup vs baseline: 1.4595x; 1.4595x over previous
"""Trainium2 Bass kernel for CRF log-likelihood (B=128, S=512, U=1024, T=48).

Strategy (data-parallel, 16 batch rows per core, no collectives):
  - Emissions scores = H @ W computed on PE (K=1024 in 8 chunks of 128),
    H streamed from HBM with U on partitions (fully contiguous reads).
  - Forward algorithm in exp space: one (49x49)@(49x16) PE matmul + one
    DVE multiply per time step.  A 49th "done" state absorbs finished rows
    (transition column = exp(end_transitions)), driven purely by per-core
    data masks, so all cores run the identical SPMD program.
  - A constant per-step normalizer exp(-C0) keeps fp32 in range; corrected
    on the host via + C0*(s_len-1).
  - The chain is split into a forward scan (steps 1..255) and an
    independent backward scan (steps 511..256) that run concurrently,
    halving the sequential latency.  Z = sum_j alpha_cut[j]*beta_cut[j].
  - Gold-path emission sum (numerator) on device via a host-built
    onehot*mask multiply + reduce against the same score tiles.
  - Tiny O(B*S) gathers of the small parameter tensors (transition/start/
    end terms of the numerator, final log/assembly) happen on the host.
"""

import os

import numpy as np

import concourse.bass as bass
import concourse.tile as tile
from concourse import bacc, mybir
from concourse.bass_utils import run_bass_kernel_spmd

B, S, U, T = 128, 512, 1024, 48
NCORES = 8
NB = B // NCORES          # 16 rows per core
NPOS = NB * S             # 8192 positions per core, pos = s*NB + b
TA = T + 1                # 49 states (48 tags + "done")
CUT = 255                 # fwd computes alpha_1..alpha_CUT, bwd beta_510..beta_CUT
C0 = 4.8                  # per-step log-space normalizer
SCHUNK = 64               # time steps per emission chunk
NCHUNK = S // SCHUNK      # 8
CPOS = SCHUNK * NB        # 1024 positions per chunk -> 2 PSUM halves of 512
NEG = -1.0e9              # pad logit; exp(NEG) == 0 in fp32
F32 = mybir.dt.float32
BF16 = mybir.dt.bfloat16
F16 = mybir.dt.float16
NEGH = -60000.0           # fp16-representable pad logit; exp() == 0

_PROGRAM = None  # compiled program cache
LAST_EXEC_NS = None
LAST_RESULT = None


def _build_program():
    nc = bacc.Bacc("TRN2", target_bir_lowering=False, debug=False,
                   enable_asserts=False)

    def din(name, shape, dt=F32):
        return nc.dram_tensor(name, list(shape), dt, kind="ExternalInput").ap()

    def dout(name, shape):
        return nc.dram_tensor(name, list(shape), F32, kind="ExternalOutput").ap()

    h = din("h", (U, S, NB), F16)  # host-pretransposed
    w = din("w", (U, TA), F16)  # 49th col zero
    lhs_fwd = din("lhs_fwd", (TA, TA), BF16)  # Ahat: [i, j]
    lhs_bwd = din("lhs_bwd", (TA, TA), BF16)  # Ahat^T
    ones_k1 = din("ones_k1", (1, TA), F16)  # [1]*48 + [-1]
    ones49 = din("ones49", (TA, 1), BF16)
    padflag = din("padflag", (1, NPOS), F16)  # {0, NEGH}
    msel = din("msel", (TA, NPOS))          # onehot(tag)*wmask, row48=0
    bias_e = din("bias_e", (TA, 1))         # [b - C0; NEG]
    bias_a0 = din("bias_a0", (TA, 1))       # [b + start; NEG]
    beta_init = din("beta_init", (TA, NB), BF16)  # [exp(end); 1]

    z_out = dout("z_out", (1, NB))
    numstage_out = dout("numstage", (TA, 2 * NCHUNK * NB))

    with tile.TileContext(nc) as tc:
        with (
            tc.tile_pool(name="consts", bufs=1) as consts,
            tc.tile_pool(name="hpool", bufs=2) as hpool,
            tc.tile_pool(name="tmp", bufs=2) as tmpp,
            tc.tile_pool(name="epsum", bufs=2, space="PSUM") as epsum,
            tc.tile_pool(name="psA", bufs=2, space="PSUM") as psA,
            tc.tile_pool(name="psB", bufs=2, space="PSUM") as psB,
            tc.tile_pool(name="psZ", bufs=1, space="PSUM") as psZ,
            tc.tile_pool(name="sA", bufs=2) as sAp,
            tc.tile_pool(name="sB", bufs=2) as sBp,
        ):
            # ---- constants into SBUF ----
            w_sb = consts.tile([128, 8 * TA], F16, tag="w_sb")
            nc.sync.dma_start(w_sb[:].rearrange("p (c t) -> p c t", c=8),
                              w.rearrange("(c p) t -> p c t", p=128))
            lhsf_sb = consts.tile([TA, TA], BF16, tag="lhsf")
            nc.sync.dma_start(lhsf_sb[:], lhs_fwd)
            lhsb_sb = consts.tile([TA, TA], BF16, tag="lhsb")
            nc.sync.dma_start(lhsb_sb[:], lhs_bwd)
            ones1_sb = consts.tile([1, TA], F16, tag="ones1")
            nc.sync.dma_start(ones1_sb[:], ones_k1)
            ones49_sb = consts.tile([TA, 1], BF16, tag="ones49v")
            nc.sync.dma_start(ones49_sb[:], ones49)
            pad_sb = consts.tile([1, NPOS], F16, tag="pad")
            nc.sync.dma_start(pad_sb[:], padflag)
            msel_sb = consts.tile([TA, NPOS], F32, tag="msel")
            nc.sync.dma_start(msel_sb[:], msel)
            bias_e_sb = consts.tile([TA, 1], F32, tag="bias_e")
            nc.sync.dma_start(bias_e_sb[:], bias_e)
            bias_a0_sb = consts.tile([TA, 1], F32, tag="bias_a0")
            nc.sync.dma_start(bias_a0_sb[:], bias_a0)
            beta0_sb = consts.tile([TA, NB], BF16, tag="beta0")
            nc.sync.dma_start(beta0_sb[:], beta_init)

            escan = consts.tile([TA, NPOS], F32, tag="escan")
            numstage_sb = consts.tile([TA, 2 * NCHUNK * NB], F32, tag="numstage")
            alpha0_sb = consts.tile([TA, NB], BF16, tag="alpha0")

            def produce_chunk(c):
                hs = hpool.tile([128, CPOS * 8], F16, tag="hs")
                for hh in range(8):
                    src = h[hh * 128:(hh + 1) * 128,
                            c * SCHUNK:(c + 1) * SCHUNK, :].rearrange(
                        "p s b -> p (s b)")
                    nc.sync.dma_start(hs[:, hh * CPOS:(hh + 1) * CPOS], src)
                for half in range(2):
                    pos0 = c * CPOS + half * 512
                    ps = epsum.tile([TA, 512], F32, tag="eps")
                    for hh in range(8):
                        nc.tensor.matmul(
                            ps[:],
                            w_sb[:, hh * TA:(hh + 1) * TA],
                            hs[:, hh * CPOS + half * 512: hh * CPOS + half * 512 + 512],
                            start=(hh == 0), stop=False)
                    nc.tensor.matmul(ps[:], ones1_sb[:],
                                     pad_sb[:, pos0:pos0 + 512],
                                     start=False, stop=True)
                    nc.scalar.activation(escan[:, pos0:pos0 + 512], ps[:],
                                         mybir.ActivationFunctionType.Exp,
                                         bias=bias_e_sb[:])
                    if c == 0 and half == 0:
                        nc.scalar.activation(alpha0_sb[:], ps[:, 0:NB],
                                             mybir.ActivationFunctionType.Exp,
                                             bias=bias_a0_sb[:])
                    t_ = tmpp.tile([TA, 512], F32, tag="nmul")
                    nc.vector.tensor_tensor(t_[:], ps[:],
                                            msel_sb[:, pos0:pos0 + 512],
                                            mybir.AluOpType.mult)
                    nc.vector.tensor_reduce(
                        numstage_sb[:, (2 * c + half) * NB:(2 * c + half + 1) * NB],
                        t_[:].rearrange("t (s b) -> t b s", b=NB),
                        mybir.AxisListType.X, mybir.AluOpType.add)

            produce_chunk(0)
            produce_chunk(NCHUNK - 1)

            # ---- the two scan chains, interleaved ----
            alpha = alpha0_sb
            beta = beta0_sb
            prefetch = {}
            for c in range(1, 4):
                lead = c * SCHUNK - 40
                prefetch.setdefault(lead, []).extend([c, NCHUNK - 1 - c])

            for i in range(CUT):
                for c in prefetch.get(i, ()):
                    produce_chunk(c)
                s_f = 1 + i
                pa = psA.tile([TA, NB], F32, tag="pa")
                nc.tensor.matmul(pa[:], lhsf_sb[:], alpha[:], start=True, stop=True)
                na = sAp.tile([TA, NB], BF16, tag="na")
                nc.vector.tensor_tensor(na[:], pa[:],
                                        escan[:, s_f * NB:(s_f + 1) * NB],
                                        mybir.AluOpType.mult)
                alpha = na

                s_b = S - 1 - i
                rb = sBp.tile([TA, NB], BF16, tag="rb")
                nc.vector.tensor_tensor(rb[:], beta[:],
                                        escan[:, s_b * NB:(s_b + 1) * NB],
                                        mybir.AluOpType.mult)
                pb = psB.tile([TA, NB], F32, tag="pb")
                nc.tensor.matmul(pb[:], lhsb_sb[:], rb[:], start=True, stop=True)
                beta = pb

            # final bwd step: s_b = CUT+1 = 256 -> beta_255
            rb = sBp.tile([TA, NB], BF16, tag="rb")
            nc.vector.tensor_tensor(rb[:], beta[:],
                                    escan[:, (CUT + 1) * NB:(CUT + 2) * NB],
                                    mybir.AluOpType.mult)
            pb = psB.tile([TA, NB], F32, tag="pb")
            nc.tensor.matmul(pb[:], lhsb_sb[:], rb[:], start=True, stop=True)

            # ---- readout: z = sum_j alpha_cut[j] * beta_cut[j] ----
            g = sAp.tile([TA, NB], BF16, tag="gamma")
            nc.vector.tensor_tensor(g[:], pb[:], alpha[:], mybir.AluOpType.mult)
            zp = psZ.tile([1, NB], F32, tag="zp")
            nc.tensor.matmul(zp[:], ones49_sb[:], g[:], start=True, stop=True)
            zsb = consts.tile([1, NB], F32, tag="zsb")
            nc.vector.tensor_copy(zsb[:], zp[:])
            nc.sync.dma_start(z_out, zsb[:])
            nc.sync.dma_start(numstage_out, numstage_sb[:])

    nc.compile()
    return nc


def _host_inputs(H, W, bb, st, en, tr, tag, s_len, w_mask):
    """Build the per-core input maps (all f32)."""
    import ml_dtypes
    BF = ml_dtypes.bfloat16
    A = np.exp(tr.astype(np.float64)).astype(np.float32)
    Ahat = np.zeros((TA, TA), np.float32)
    Ahat[:T, :T] = A
    Ahat[:T, T] = np.exp(en).astype(np.float32)
    Ahat[T, T] = 1.0

    beta_init = np.zeros((TA, NB), np.float32)
    beta_init[:T, :] = np.exp(en).astype(np.float32)[:, None]
    beta_init[T, :] = 1.0
    NEGb = np.float32(np.float16(NEGH))  # fp16 pad logit (exact cancel)

    Wp = np.zeros((U, TA), np.float16)
    Wp[:, :T] = W.astype(np.float16)
    ones_k1 = np.ones((1, TA), np.float16)
    ones_k1[0, T] = -1.0
    shared = {
        "w": Wp,
        "lhs_fwd": Ahat.astype(BF),
        "lhs_bwd": np.ascontiguousarray(Ahat.T).astype(BF),
        "ones_k1": ones_k1,
        "ones49": np.ones((TA, 1), BF),
        "bias_e": np.concatenate([(bb - C0).astype(np.float32),
                                  [NEGb]]).reshape(TA, 1),
        "bias_a0": np.concatenate([(bb + st).astype(np.float32),
                                   [np.float32(NEG)]]).reshape(TA, 1),
        "beta_init": beta_init.astype(BF),
    }

    s_idx = np.arange(S)
    in_maps = []
    for k in range(NCORES):
        rows = slice(k * NB, (k + 1) * NB)
        tag_l = tag[rows]            # (NB, S)
        len_l = s_len[rows]          # (NB,)
        wm_l = w_mask[rows]          # (NB, S)
        pad = (s_idx[None, :] >= len_l[:, None])          # (NB, S)
        padflag = np.where(pad, NEGb, np.float32(0.0)).T.reshape(1, NPOS).astype(np.float16)
        msel3 = np.zeros((TA, S, NB), np.float32)
        msel3[tag_l.T, s_idx[:, None], np.arange(NB)[None, :]] = wm_l.T
        im = dict(shared)
        im["h"] = np.ascontiguousarray(H[rows].transpose(2, 1, 0).astype(np.float16))
        im["padflag"] = np.ascontiguousarray(padflag)
        im["msel"] = np.ascontiguousarray(msel3.reshape(TA, NPOS))
        in_maps.append(im)
    return in_maps


def kernel(H, W, b, start_transitions, end_transitions, transitions,
           tag, s_len, w_mask):
    global _PROGRAM
    H = np.asarray(H, np.float32)
    W = np.asarray(W, np.float32)
    bb = np.asarray(b, np.float32)
    st = np.asarray(start_transitions, np.float32)
    en = np.asarray(end_transitions, np.float32)
    tr = np.asarray(transitions, np.float32)
    tag = np.asarray(tag)
    s_len = np.asarray(s_len)
    w_mask = np.asarray(w_mask, np.float32)

    if _PROGRAM is None:
        _PROGRAM = _build_program()
    nc = _PROGRAM

    in_maps = _host_inputs(H, W, bb, st, en, tr, tag, s_len, w_mask)
    trace = bool(int(os.environ.get("KERNEL_TRACE", "0")))
    r = run_bass_kernel_spmd(nc, in_maps, list(range(NCORES)), trace=trace,
                             tmpdir=os.environ.get("KERNEL_TRACE_DIR") or None)
    global LAST_EXEC_NS, LAST_RESULT
    LAST_RESULT = r
    LAST_EXEC_NS = r.exec_time_ns
    res = r.results

    z = np.concatenate([np.asarray(r["z_out"]).reshape(NB) for r in res])
    numstage = np.stack([np.asarray(r["numstage"]) for r in res])  # (NC, TA, 32*NB)

    # ---- host assembly ----
    logZ = np.log(z.astype(np.float64)) + C0 * (s_len.astype(np.float64) - 1)
    num_emit = (numstage.reshape(NCORES, TA, 2 * NCHUNK, NB).sum(axis=(1, 2))
                .reshape(B).astype(np.float64))
    bidx = np.arange(B)
    num = (st[tag[:, 0]].astype(np.float64)
           + num_emit
           + (bb[tag].astype(np.float64) * w_mask).sum(axis=1)
           + (tr[tag[:, :-1], tag[:, 1:]].astype(np.float64) * w_mask[:, 1:]).sum(axis=1)
           + en[tag[bidx, s_len - 1]].astype(np.float64))
    return (num - logZ).astype(np.float32)
